# revision 13
# baseline (speedup 1.0000x reference)
"""Trainium2 Bass kernel for nn_HGCEncoder (gnn_message_passing).

Reference computation (H=4 sequential heads, B=16, S=2048, D=128):
    g = cor_matrix.mean(-1)[event_type - 1]          # [B, S]  (host prep)
    x = output
    for h in range(H):
        z    = elu(x @ W3[h].T + b3[h])              # [B, S, D]
        corr = local_cor @ z                         # banded [1,S,S] @ [B,S,D]
        sum_seq  += corr
        sum_head += einsum('bsd,bs->bd', corr, g)
        x = corr
    return (sum_seq, sum_head)

Sharding: data-parallel over B across 8 cores (2 batches per core).

Device-side layout is "form 1": all big tensors live as [D=128 partitions,
S free] transposed ("xT"), so that
  - GEMM1 z[s_blk, e] = matmul(lhsT=xT[:, s_blk], rhs=W3[h].T)   (K=d)
  - stage2 corrT[d, i_blk] = sum_jb matmul(lhsT=z[jb], rhs=bandT_blk)  (K=j)
chain with no transposes: stage2's output corrT is directly the next
head's GEMM1 stationary operand.

local_cor is analyzed on the host into block-sparse form: for each
128-row block ib, the list of 128-col blocks jb with any nonzero, each
mapped to a deduplicated [128,128] constant tile (transposed to [j,i]).
For the reference band (width-64 latest-k mask) this yields exactly 2
unique tiles. ELU is computed exactly as max(x, exp(min(x,0)) - 1).
"""

import os
import sys

for _p in ("/opt/trn_rl_repo", "/root/.axon_site/_ro/trn_rl_repo"):
    if os.path.isdir(_p) and _p not in sys.path:
        sys.path.insert(0, _p)

import numpy as np

import concourse.bacc as bacc
import concourse.mybir as mybir
import concourse.tile as tile
from concourse import bass_utils

F32 = mybir.dt.float32
BF16 = mybir.dt.bfloat16
AFT = mybir.ActivationFunctionType
ALU = mybir.AluOpType

N_CORES = 8
B, S, D, H = 16, 2048, 128, 4
BL = B // N_CORES          # batches per core = 2
NB = S // 128              # 16 s-blocks of 128


def _analyze_local_cor(local_cor):
    """Block-sparse analysis of local_cor[0] ([S,S]).

    Returns (blocks, bands_np): blocks[ib] = [(jb, uidx), ...];
    bands_np [U,128,128] = deduped transposed ([j,i]) block constants.
    """
    lc = np.ascontiguousarray(np.asarray(local_cor).reshape(S, S), dtype=np.float32)
    uniq = {}
    tiles = []
    blocks = []
    for ib in range(NB):
        row = []
        rb = lc[ib * 128:(ib + 1) * 128]
        for jb in range(NB):
            blk = rb[:, jb * 128:(jb + 1) * 128]
            if np.any(blk):
                key = blk.tobytes()
                u = uniq.get(key)
                if u is None:
                    u = len(tiles)
                    uniq[key] = u
                    tiles.append(np.ascontiguousarray(blk.T))
                row.append((jb, u))
        blocks.append(row)
    bands_np = np.stack(tiles) if tiles else np.zeros((1, 128, 128), np.float32)
    return blocks, bands_np


def _is_canonical_band(blocks):
    """True when blocks has the translation-invariant 2-diagonal pattern:
    blocks[ib] == [(ib, u0), (ib+1, u1)] for ib < NB-1, [(NB-1, u0)] last."""
    if len(blocks) != NB or len(blocks[0]) != 2:
        return False
    u0 = blocks[0][0][1]
    u1 = blocks[0][1][1]
    for ib in range(NB - 1):
        if blocks[ib] != [(ib, u0), (ib + 1, u1)]:
            return False
    return blocks[NB - 1] == [(NB - 1, u0)]


_PROGRAM_CACHE = {}


def _axon_device_reset():
    """Best-effort recovery if a previous run left a core wedged."""
    try:
        import ctypes
        import jax
        jax.devices()
        lib = ctypes.CDLL("/opt/axon/libaxon_pjrt.so")
        lib.axon_reset.restype = ctypes.c_int64
        lib.axon_reset()
    except Exception:
        pass


def _build_program(blocks, n_uniq, b3_nonzero):
    """Build + compile the per-core Bass program (same NEFF on all cores)."""
    nc = bacc.Bacc("TRN2", target_bir_lowering=False, debug=False)

    xT_d = nc.dram_tensor("xT", [BL, D, S], BF16, kind="ExternalInput").ap()
    w3t_d = nc.dram_tensor("w3t", [H, D, D], BF16, kind="ExternalInput").ap()
    bands_d = nc.dram_tensor("bands", [n_uniq, 128, 128], BF16, kind="ExternalInput").ap()
    merged = _is_canonical_band(blocks)
    if merged:
        # [T2T | T1T] side by side: one N=256 matmul per stationary z block
        bandpair_d = nc.dram_tensor("bandpair", [128, 256], BF16, kind="ExternalInput").ap()
    if b3_nonzero:
        b3_d = nc.dram_tensor("b3t4", [H, 128, 512], F32, kind="ExternalInput").ap()
    corr_d = nc.dram_tensor("corrT", [H, BL, D, S], BF16, kind="ExternalOutput").ap()

    with tile.TileContext(nc) as tc:
        with (
            tc.tile_pool(name="consts", bufs=1) as consts,
            tc.tile_pool(name="xbuf", bufs=3) as xpool,
            tc.tile_pool(name="zbuf", bufs=2) as zpool,
            tc.tile_pool(name="scratch", bufs=3) as spool,
            tc.tile_pool(name="zp", bufs=2, space="PSUM") as zppool,
            tc.tile_pool(name="cp", bufs=2, space="PSUM") as cppool,
        ):
            # ---- constant loads ----
            w3t_sb = consts.tile([D, H, D], BF16, tag="w3t", name="w3t_sb")
            nc.sync.dma_start(w3t_sb[:], w3t_d.rearrange("h d e -> d h e"))
            bands_sb = consts.tile([128, n_uniq, 128], BF16, tag="bands", name="bands_sb")
            nc.sync.dma_start(bands_sb[:], bands_d.rearrange("u j i -> j u i"))
            if merged:
                bandpair_sb = consts.tile([128, 256], BF16, tag="bandpair", name="bandpair_sb")
                nc.sync.dma_start(bandpair_sb[:], bandpair_d[:])
                zero_sb = consts.tile([1, 512], BF16, tag="zero_sb", name="zero_sb")
                nc.vector.memset(zero_sb[:], 0.0)
            if b3_nonzero:
                b3_sb = consts.tile([128, H, 512], F32, tag="b3", name="b3_sb")
                nc.sync.dma_start(b3_sb[:], b3_d.rearrange("h p e -> p h e"))

            xbuf0 = []
            for b in range(BL):
                xb = xpool.tile([D, S], BF16, tag=f"x_{b}", name=f"xb_{b}")
                nc.sync.dma_start(xb[:, :1024], xT_d[b][:, :1024])
                nc.sync.dma_start(xb[:, 1024:], xT_d[b][:, 1024:])
                xbuf0.append(xb)

            # ---- main per-batch pipeline ----
            # stage2 emission is jb-major: all matmuls consuming z[jb] are
            # adjacent (stationary reuse). Precompute, per jb, the list of
            # (ib, uidx, is_first, is_last) it contributes to.
            by_jb = [[] for _ in range(NB)]
            for ib in range(NB):
                lst = blocks[ib]
                for idx, (jb, u) in enumerate(lst):
                    by_jb[jb].append((ib, u, idx == 0, idx == len(lst) - 1))
            # i-block group (of 4) is fully accumulated once every ib in it
            # has seen its last contribution
            grp_done_at_jb = [0] * (NB // 8)
            for jb in range(NB):
                for (ib, u, fi, la) in by_jb[jb]:
                    if la:
                        grp_done_at_jb[ib // 8] = max(grp_done_at_jb[ib // 8], jb)

            x_cur_b = list(xbuf0)
            for h in range(H):
                for b in range(BL):
                    x_cur = x_cur_b[b]
                    # GEMM1 + ELU -> z_sb [128(s_local), S(=16 blocks of e)]
                    z_sb = zpool.tile([128, S], BF16, tag=f"z_{b}", name=f"z_{b}")
                    for sg in range(NB // 8):       # groups of 8 s-blocks
                        zp = zppool.tile([128, 1024], F32, tag="zp", name="zp_t")
                        for k in range(8):
                            sb_i = sg * 8 + k
                            nc.tensor.matmul(
                                zp[:, k * 128:(k + 1) * 128],
                                lhsT=x_cur[:, sb_i * 128:(sb_i + 1) * 128],
                                rhs=w3t_sb[:, h, :],
                                start=True, stop=True,
                            )
                        if b3_nonzero:
                            nc.vector.tensor_add(zp[:, :512], zp[:, :512], b3_sb[:, h, :])
                            nc.vector.tensor_add(zp[:, 512:], zp[:, 512:], b3_sb[:, h, :])
                        # elu(x) = max(x, min(exp(x), 1) - 1); exp saturates to
                        # +inf on overflow which the min clamps. 512-wide halves
                        # so the chain starts after 4 GEMM1 matmuls, not 8.
                        for hf in range(2):
                            hs = slice(hf * 512, (hf + 1) * 512)
                            zslc = z_sb[:, sg * 1024 + hf * 512: sg * 1024 + (hf + 1) * 512]
                            e_sb = spool.tile([128, 512], F32, tag="elu_e", name="elu_e")
                            nc.scalar.activation(e_sb[:], zp[:, hs], AFT.Exp)
                            u_sb = spool.tile([128, 512], BF16, tag="elu_u", name="elu_u")
                            nc.vector.tensor_scalar(u_sb[:], e_sb[:], 1.0, -1.0, ALU.min, ALU.add)
                            nc.vector.tensor_max(zslc, u_sb[:], zp[:, hs])

                    # stage2: banded matmul -> corrT [d, i]; also accumulate
                    c_out = xpool.tile([D, S], BF16, tag=f"x_{b}", name=f"xb_{b}")
                    if merged:
                        GW = 1024          # psum group width
                        NG = S // GW       # 2 groups
                        cps = []
                        for ig in range(NG):
                            cpt = cppool.tile([128, GW], F32, tag="cp", name="cp_t")
                            cps.append(cpt)
                            for half in range(GW // 512):
                                nc.tensor.matmul(
                                    cpt[:, half * 512:(half + 1) * 512],
                                    lhsT=zero_sb[:, :128], rhs=zero_sb[:],
                                    start=True, stop=False, skip_group_check=True,
                                )
                        def _mm(lo, wid, zjb, rlo, last):
                            ig, off = lo // GW, lo % GW
                            nc.tensor.matmul(
                                cps[ig][:, off:off + wid],
                                lhsT=z_sb[:, zjb * 128:(zjb + 1) * 128],
                                rhs=bandpair_sb[:, rlo:rlo + wid],
                                start=False, stop=last, skip_group_check=True,
                            )
                        # jb=0 contributes T1 only (cols 0:128)
                        _mm(0, 128, 0, 128, False)
                        for jb in range(1, NB):
                            lo = (jb - 1) * 128
                            if lo % 512 != 384 and lo // GW == (lo + 255) // GW:
                                _mm(lo, 256, jb, 0, jb == NB - 1)
                            else:  # span crosses a psum bank/group boundary: split
                                _mm(lo, 128, jb, 0, lo // GW != (lo + 255) // GW)
                                _mm(lo + 128, 128, jb, 128, False)
                            # drain a group as soon as its last region is done:
                            # group ig completes at jb == (ig+1)*8 (T2 of its
                            # last region) or at jb == NB-1 for the final group
                            for ig in range(NG):
                                done_jb = (ig + 1) * (GW // 128) if ig < NG - 1 else NB - 1
                                if jb == done_jb:
                                    oslc = slice(ig * GW, (ig + 1) * GW)
                                    nc.any.tensor_copy(c_out[:, oslc], cps[ig][:])
                                    nc.sync.dma_start(corr_d[h, b][:, oslc], c_out[:, oslc])
                    else:
                        cps = [None] * (NB // 8)
                        for jb in range(NB):
                            for (ib, u, fi, la) in by_jb[jb]:
                                ig = ib // 8
                                if cps[ig] is None:
                                    cps[ig] = cppool.tile([128, 1024], F32, tag="cp", name="cp_t")
                                k = ib % 8
                                nc.tensor.matmul(
                                    cps[ig][:, k * 128:(k + 1) * 128],
                                    lhsT=z_sb[:, jb * 128:(jb + 1) * 128],
                                    rhs=bands_sb[:, u, :],
                                    start=fi, stop=la,
                                )
                            # drain any group that completed at this jb
                            for ig in range(NB // 8):
                                if grp_done_at_jb[ig] == jb and cps[ig] is not None:
                                    cp = cps[ig]
                                    oslc = slice(ig * 1024, (ig + 1) * 1024)
                                    nc.any.tensor_copy(c_out[:, oslc], cp[:])
                    if not merged:
                        nc.sync.dma_start(corr_d[h, b], c_out[:])
                    x_cur_b[b] = c_out


    nc.compile()
    return nc


def _get_program(blocks, n_uniq, b3_nonzero):
    key = (tuple(tuple(r) for r in blocks), n_uniq, b3_nonzero)
    prog = _PROGRAM_CACHE.get(key)
    if prog is None:
        prog = _build_program(blocks, n_uniq, b3_nonzero)
        _PROGRAM_CACHE[key] = prog
    return prog


def run(output, local_cor, cor_matrix, event_type, W3, b3, trace=False, tmpdir=None):
    output = np.ascontiguousarray(np.asarray(output), dtype=np.float32)
    W3 = np.asarray(W3, dtype=np.float32)
    b3 = np.asarray(b3, dtype=np.float32)
    cor_matrix = np.asarray(cor_matrix, dtype=np.float32)
    event_type = np.asarray(event_type)

    assert output.shape == (B, S, D), output.shape
    # host prep: gather of per-type means (trivial), sharding, transposes
    g = cor_matrix.mean(-1)[event_type.astype(np.int64) - 1].astype(np.float32)
    blocks, bands_np = _analyze_local_cor(local_cor)
    b3_nonzero = bool(np.any(b3))

    nc = _get_program(blocks, len(bands_np), b3_nonzero)

    import ml_dtypes
    bf16 = ml_dtypes.bfloat16
    w3t_np = np.ascontiguousarray(W3.transpose(0, 2, 1)).astype(bf16)  # [H, d, e]
    bands_np = bands_np.astype(bf16)
    in_maps = []
    for c in range(N_CORES):
        bs = slice(c * BL, (c + 1) * BL)
        m = {
            "xT": np.ascontiguousarray(output[bs].transpose(0, 2, 1)).astype(bf16),
            "w3t": w3t_np,
            "bands": bands_np,
        }
        if _is_canonical_band(blocks):
            u0 = blocks[0][0][1]
            u1 = blocks[0][1][1]
            m["bandpair"] = np.ascontiguousarray(
                np.concatenate([bands_np[u1], bands_np[u0]], axis=1))
        if b3_nonzero:
            m["b3t4"] = np.ascontiguousarray(
                np.broadcast_to(np.tile(b3, (1, 4))[:, None, :], (H, 128, 512))
            ).astype(np.float32)
        in_maps.append(m)

    _axon_device_reset()
    res = bass_utils.run_bass_kernel_spmd(
        nc, in_maps, core_ids=list(range(N_CORES)), trace=trace, tmpdir=tmpdir,
    )

    # gather/unshard: per-head corrT [H, BL, D, S] bf16 per core.
    # sum over heads + g-weighted reduction are part of the unshard.
    corr = np.concatenate([r["corrT"] for r in res.results], axis=1)  # [H, B, D, S]
    sum_seqT = corr.astype(np.float32).sum(axis=0)                    # [B, D, S]
    sum_seq = np.ascontiguousarray(sum_seqT.transpose(0, 2, 1))       # [B, S, D]
    sum_head = np.einsum("bds,bs->bd", sum_seqT, g).astype(np.float32)
    return (sum_seq, sum_head), res


def kernel(output, local_cor, cor_matrix, event_type, W3, b3):
    (sum_seq, sum_head), _ = run(output, local_cor, cor_matrix, event_type, W3, b3)
    return (sum_seq, sum_head)


# revision 15
# speedup vs baseline: 1.2950x; 1.2950x over previous
"""Trainium2 Bass kernel for nn_HGCEncoder (gnn_message_passing).

Reference computation (H=4 sequential heads, B=16, S=2048, D=128):
    g = cor_matrix.mean(-1)[event_type - 1]          # [B, S]  (host prep)
    x = output
    for h in range(H):
        z    = elu(x @ W3[h].T + b3[h])              # [B, S, D]
        corr = local_cor @ z                         # banded [1,S,S] @ [B,S,D]
        sum_seq  += corr
        sum_head += einsum('bsd,bs->bd', corr, g)
        x = corr
    return (sum_seq, sum_head)

Sharding: data-parallel over B across 8 cores (2 batches per core).

Device-side layout is "form 1": all big tensors live as [D=128 partitions,
S free] transposed ("xT"), so that
  - GEMM1 z[s_blk, e] = matmul(lhsT=xT[:, s_blk], rhs=W3[h].T)   (K=d)
  - stage2 corrT[d, i_blk] = sum_jb matmul(lhsT=z[jb], rhs=bandT_blk)  (K=j)
chain with no transposes: stage2's output corrT is directly the next
head's GEMM1 stationary operand.

local_cor is analyzed on the host into block-sparse form: for each
128-row block ib, the list of 128-col blocks jb with any nonzero, each
mapped to a deduplicated [128,128] constant tile (transposed to [j,i]).
For the reference band (width-64 latest-k mask) this yields exactly 2
unique tiles. ELU is computed exactly as max(x, exp(min(x,0)) - 1).
"""

import os
import sys

for _p in ("/opt/trn_rl_repo", "/root/.axon_site/_ro/trn_rl_repo"):
    if os.path.isdir(_p) and _p not in sys.path:
        sys.path.insert(0, _p)

import numpy as np

import concourse.bacc as bacc
import concourse.mybir as mybir
import concourse.tile as tile
from concourse import bass_utils

F32 = mybir.dt.float32
BF16 = mybir.dt.bfloat16
AFT = mybir.ActivationFunctionType
ALU = mybir.AluOpType

N_CORES = 8
B, S, D, H = 16, 2048, 128, 4
BL = B // N_CORES          # batches per core = 2
NB = S // 128              # 16 s-blocks of 128


def _analyze_local_cor(local_cor):
    """Block-sparse analysis of local_cor[0] ([S,S]).

    Returns (blocks, bands_np): blocks[ib] = [(jb, uidx), ...];
    bands_np [U,128,128] = deduped transposed ([j,i]) block constants.
    """
    lc = np.ascontiguousarray(np.asarray(local_cor).reshape(S, S), dtype=np.float32)
    uniq = {}
    tiles = []
    blocks = []
    for ib in range(NB):
        row = []
        rb = lc[ib * 128:(ib + 1) * 128]
        for jb in range(NB):
            blk = rb[:, jb * 128:(jb + 1) * 128]
            if np.any(blk):
                key = blk.tobytes()
                u = uniq.get(key)
                if u is None:
                    u = len(tiles)
                    uniq[key] = u
                    tiles.append(np.ascontiguousarray(blk.T))
                row.append((jb, u))
        blocks.append(row)
    bands_np = np.stack(tiles) if tiles else np.zeros((1, 128, 128), np.float32)
    return blocks, bands_np


def _is_canonical_band(blocks):
    """True when blocks has the translation-invariant 2-diagonal pattern:
    blocks[ib] == [(ib, u0), (ib+1, u1)] for ib < NB-1, [(NB-1, u0)] last."""
    if len(blocks) != NB or len(blocks[0]) != 2:
        return False
    u0 = blocks[0][0][1]
    u1 = blocks[0][1][1]
    for ib in range(NB - 1):
        if blocks[ib] != [(ib, u0), (ib + 1, u1)]:
            return False
    return blocks[NB - 1] == [(NB - 1, u0)]


_PROGRAM_CACHE = {}


def _axon_device_reset():
    """Best-effort recovery if a previous run left a core wedged."""
    try:
        import ctypes
        import jax
        jax.devices()
        lib = ctypes.CDLL("/opt/axon/libaxon_pjrt.so")
        lib.axon_reset.restype = ctypes.c_int64
        lib.axon_reset()
    except Exception:
        pass


def _build_program(blocks, n_uniq, b3_nonzero):
    """Build + compile the per-core Bass program (same NEFF on all cores)."""
    nc = bacc.Bacc("TRN2", target_bir_lowering=False, debug=False)

    xT_d = nc.dram_tensor("xT", [BL, D, S], BF16, kind="ExternalInput").ap()
    w3t_d = nc.dram_tensor("w3t", [H, D, D], BF16, kind="ExternalInput").ap()
    bands_d = nc.dram_tensor("bands", [n_uniq, 128, 128], BF16, kind="ExternalInput").ap()
    merged = _is_canonical_band(blocks)
    if merged:
        # [T2T | T1T] side by side: one N=256 matmul per stationary z block
        bandpair_d = nc.dram_tensor("bandpair", [128, 256], BF16, kind="ExternalInput").ap()
    if b3_nonzero:
        b3_d = nc.dram_tensor("b3t4", [H, 128, 512], F32, kind="ExternalInput").ap()
    corr_d = nc.dram_tensor("corrT", [H, BL, D, S], BF16, kind="ExternalOutput").ap()

    with tile.TileContext(nc) as tc:
        with (
            tc.tile_pool(name="consts", bufs=1) as consts,
            tc.tile_pool(name="xbuf", bufs=3) as xpool,
            tc.tile_pool(name="zbuf", bufs=2) as zpool,
            tc.tile_pool(name="scratch", bufs=3) as spool,
            tc.tile_pool(name="zp", bufs=2, space="PSUM") as zppool,
            tc.tile_pool(name="cp", bufs=2, space="PSUM") as cppool,
        ):
            # ---- constant loads ----
            w3t_sb = consts.tile([D, H, D], BF16, tag="w3t", name="w3t_sb")
            nc.sync.dma_start(w3t_sb[:], w3t_d.rearrange("h d e -> d h e"))
            bands_sb = consts.tile([128, n_uniq, 128], BF16, tag="bands", name="bands_sb")
            nc.sync.dma_start(bands_sb[:], bands_d.rearrange("u j i -> j u i"))
            if merged:
                bandpair_sb = consts.tile([128, 256], BF16, tag="bandpair", name="bandpair_sb")
                nc.sync.dma_start(bandpair_sb[:], bandpair_d[:])
                zero_sb = consts.tile([1, 512], BF16, tag="zero_sb", name="zero_sb")
                nc.vector.memset(zero_sb[:], 0.0)
            if b3_nonzero:
                b3_sb = consts.tile([128, H, 512], F32, tag="b3", name="b3_sb")
                nc.sync.dma_start(b3_sb[:], b3_d.rearrange("h p e -> p h e"))

            xbuf0 = []
            for b in range(BL):
                xb = xpool.tile([D, S], BF16, tag=f"x_{b}", name=f"xb_{b}")
                nc.sync.dma_start(xb[:, :1024], xT_d[b][:, :1024])
                nc.sync.dma_start(xb[:, 1024:], xT_d[b][:, 1024:])
                xbuf0.append(xb)

            # ---- main per-batch pipeline ----
            # stage2 emission is jb-major: all matmuls consuming z[jb] are
            # adjacent (stationary reuse). Precompute, per jb, the list of
            # (ib, uidx, is_first, is_last) it contributes to.
            by_jb = [[] for _ in range(NB)]
            for ib in range(NB):
                lst = blocks[ib]
                for idx, (jb, u) in enumerate(lst):
                    by_jb[jb].append((ib, u, idx == 0, idx == len(lst) - 1))
            # i-block group (of 4) is fully accumulated once every ib in it
            # has seen its last contribution
            grp_done_at_jb = [0] * (NB // 8)
            for jb in range(NB):
                for (ib, u, fi, la) in by_jb[jb]:
                    if la:
                        grp_done_at_jb[ib // 8] = max(grp_done_at_jb[ib // 8], jb)

            x_cur_b = list(xbuf0)
            for h in range(H):
                for b in range(BL):
                    x_cur = x_cur_b[b]
                    # GEMM1 + ELU -> z_sb [128(s_local), S(=16 blocks of e)]
                    z_sb = zpool.tile([128, S], BF16, tag=f"z_{b}", name=f"z_{b}")
                    for sg in range(NB // 8):       # groups of 8 s-blocks
                        zp = zppool.tile([128, 1024], F32, tag="zp", name="zp_t")
                        for k in range(8):
                            sb_i = sg * 8 + k
                            nc.tensor.matmul(
                                zp[:, k * 128:(k + 1) * 128],
                                lhsT=x_cur[:, sb_i * 128:(sb_i + 1) * 128],
                                rhs=w3t_sb[:, h, :],
                                start=True, stop=True,
                            )
                        if b3_nonzero:
                            nc.vector.tensor_add(zp[:, :512], zp[:, :512], b3_sb[:, h, :])
                            nc.vector.tensor_add(zp[:, 512:], zp[:, 512:], b3_sb[:, h, :])
                        zslc = z_sb[:, sg * 1024:(sg + 1) * 1024]
                        # elu(x) = max(x, min(exp(x), 1) - 1); exp saturates to
                        # +inf on overflow which the min clamps.
                        e_sb = spool.tile([128, 1024], F32, tag="elu_e", name="elu_e")
                        nc.scalar.activation(e_sb[:], zp[:], AFT.Exp)
                        u_sb = spool.tile([128, 1024], BF16, tag="elu_u", name="elu_u")
                        nc.vector.tensor_scalar(u_sb[:], e_sb[:], 1.0, -1.0, ALU.min, ALU.add)
                        nc.vector.tensor_max(zslc, u_sb[:], zp[:])

                    # stage2: banded matmul -> corrT [d, i]; also accumulate
                    c_out = xpool.tile([D, S], BF16, tag=f"x_{b}", name=f"xb_{b}")
                    if merged:
                        GW = 1024          # psum group width
                        NG = S // GW       # 2 groups
                        cps = []
                        for ig in range(NG):
                            cpt = cppool.tile([128, GW], F32, tag="cp", name="cp_t")
                            cps.append(cpt)
                            for half in range(GW // 512):
                                nc.tensor.matmul(
                                    cpt[:, half * 512:(half + 1) * 512],
                                    lhsT=zero_sb[:, :128], rhs=zero_sb[:],
                                    start=True, stop=False, skip_group_check=True,
                                )
                        def _mm(lo, wid, zjb, rlo, last):
                            ig, off = lo // GW, lo % GW
                            nc.tensor.matmul(
                                cps[ig][:, off:off + wid],
                                lhsT=z_sb[:, zjb * 128:(zjb + 1) * 128],
                                rhs=bandpair_sb[:, rlo:rlo + wid],
                                start=False, stop=last, skip_group_check=True,
                            )
                        # jb=0 contributes T1 only (cols 0:128)
                        _mm(0, 128, 0, 128, False)
                        for jb in range(1, NB):
                            lo = (jb - 1) * 128
                            if lo % 512 != 384 and lo // GW == (lo + 255) // GW:
                                _mm(lo, 256, jb, 0, jb == NB - 1)
                            else:  # span crosses a psum bank/group boundary: split
                                _mm(lo, 128, jb, 0, lo // GW != (lo + 255) // GW)
                                _mm(lo + 128, 128, jb, 128, False)
                            # drain a group as soon as its last region is done:
                            # group ig completes at jb == (ig+1)*8 (T2 of its
                            # last region) or at jb == NB-1 for the final group
                            for ig in range(NG):
                                done_jb = (ig + 1) * (GW // 128) if ig < NG - 1 else NB - 1
                                if jb == done_jb:
                                    oslc = slice(ig * GW, (ig + 1) * GW)
                                    # balance psum->sbuf drains: DVE is the
                                    # saturated engine, so bias copies to ACT
                                    if (h * 2 + b) % 4 == 0 and ig == 0:
                                        nc.vector.tensor_copy(c_out[:, oslc], cps[ig][:])
                                    else:
                                        nc.scalar.copy(c_out[:, oslc], cps[ig][:])
                                    nc.sync.dma_start(corr_d[h, b][:, oslc], c_out[:, oslc])
                    else:
                        cps = [None] * (NB // 8)
                        for jb in range(NB):
                            for (ib, u, fi, la) in by_jb[jb]:
                                ig = ib // 8
                                if cps[ig] is None:
                                    cps[ig] = cppool.tile([128, 1024], F32, tag="cp", name="cp_t")
                                k = ib % 8
                                nc.tensor.matmul(
                                    cps[ig][:, k * 128:(k + 1) * 128],
                                    lhsT=z_sb[:, jb * 128:(jb + 1) * 128],
                                    rhs=bands_sb[:, u, :],
                                    start=fi, stop=la,
                                )
                            # drain any group that completed at this jb
                            for ig in range(NB // 8):
                                if grp_done_at_jb[ig] == jb and cps[ig] is not None:
                                    cp = cps[ig]
                                    oslc = slice(ig * 1024, (ig + 1) * 1024)
                                    nc.any.tensor_copy(c_out[:, oslc], cp[:])
                    if not merged:
                        nc.sync.dma_start(corr_d[h, b], c_out[:])
                    x_cur_b[b] = c_out


    nc.compile()
    return nc


def _get_program(blocks, n_uniq, b3_nonzero):
    key = (tuple(tuple(r) for r in blocks), n_uniq, b3_nonzero)
    prog = _PROGRAM_CACHE.get(key)
    if prog is None:
        prog = _build_program(blocks, n_uniq, b3_nonzero)
        _PROGRAM_CACHE[key] = prog
    return prog


def run(output, local_cor, cor_matrix, event_type, W3, b3, trace=False, tmpdir=None):
    output = np.ascontiguousarray(np.asarray(output), dtype=np.float32)
    W3 = np.asarray(W3, dtype=np.float32)
    b3 = np.asarray(b3, dtype=np.float32)
    cor_matrix = np.asarray(cor_matrix, dtype=np.float32)
    event_type = np.asarray(event_type)

    assert output.shape == (B, S, D), output.shape
    # host prep: gather of per-type means (trivial), sharding, transposes
    g = cor_matrix.mean(-1)[event_type.astype(np.int64) - 1].astype(np.float32)
    blocks, bands_np = _analyze_local_cor(local_cor)
    b3_nonzero = bool(np.any(b3))

    nc = _get_program(blocks, len(bands_np), b3_nonzero)

    import ml_dtypes
    bf16 = ml_dtypes.bfloat16
    w3t_np = np.ascontiguousarray(W3.transpose(0, 2, 1)).astype(bf16)  # [H, d, e]
    bands_np = bands_np.astype(bf16)
    in_maps = []
    for c in range(N_CORES):
        bs = slice(c * BL, (c + 1) * BL)
        m = {
            "xT": np.ascontiguousarray(output[bs].transpose(0, 2, 1)).astype(bf16),
            "w3t": w3t_np,
            "bands": bands_np,
        }
        if _is_canonical_band(blocks):
            u0 = blocks[0][0][1]
            u1 = blocks[0][1][1]
            m["bandpair"] = np.ascontiguousarray(
                np.concatenate([bands_np[u1], bands_np[u0]], axis=1))
        if b3_nonzero:
            m["b3t4"] = np.ascontiguousarray(
                np.broadcast_to(np.tile(b3, (1, 4))[:, None, :], (H, 128, 512))
            ).astype(np.float32)
        in_maps.append(m)

    _axon_device_reset()
    res = bass_utils.run_bass_kernel_spmd(
        nc, in_maps, core_ids=list(range(N_CORES)), trace=trace, tmpdir=tmpdir,
    )

    # gather/unshard: per-head corrT [H, BL, D, S] bf16 per core.
    # sum over heads + g-weighted reduction are part of the unshard.
    corr = np.concatenate([r["corrT"] for r in res.results], axis=1)  # [H, B, D, S]
    sum_seqT = corr.astype(np.float32).sum(axis=0)                    # [B, D, S]
    sum_seq = np.ascontiguousarray(sum_seqT.transpose(0, 2, 1))       # [B, S, D]
    sum_head = np.einsum("bds,bs->bd", sum_seqT, g).astype(np.float32)
    return (sum_seq, sum_head), res


def kernel(output, local_cor, cor_matrix, event_type, W3, b3):
    (sum_seq, sum_head), _ = run(output, local_cor, cor_matrix, event_type, W3, b3)
    return (sum_seq, sum_head)
